# revision 18
# baseline (speedup 1.0000x reference)
"""Trainium2 Bass kernel for nn_KanBoard768 (KAN network forward pass).

Data-parallel across 8 NeuronCores: batch 32768 -> 4096 rows/core, weights
replicated, no collectives.

v5 design (from v4):
- All [128,128] stationaries in fp16 (fast weight load; same 11-bit mantissa
  as float32r). kan1 tail features fp16; v-features f32r (DVE input fidelity).
- silu base paths FOLDED into the centered v-polynomials (deg-3 fit for kan1
  over the true h1 range, deg-4 for kan2) -- removes all Silu activations and
  their matmuls.
- squares on scalar (Square reads PSUM directly); cubes/quartic on gpsimd.
- kan2: per-edge WINDOWED truncated-power pages (9 of 12; window start from
  the host subsample, below-window shifts folded exactly into the q-poly,
  above-window shifts dropped) + q1..q4 poly stationaries.
- kan2 stage software-pipelined TWO TILES BEHIND: the f2 page generation for
  tile t runs on the DVE during tile t+1 (inputs ready, no stall), and the
  kan2 thin matmuls issue during tile t+2 between ft and kan1, so neither
  the PE nor the DVE ever waits on the other within an iteration.
"""

import numpy as np

# --- problem constants (hardcoded; kernel.py must be self-contained) ---
GRID_SIZE, SPLINE_ORDER = 5, 3
H = 2.0 / GRID_SIZE                    # 0.4
G0 = -SPLINE_ORDER * H - 1.0           # -2.2
INV_H = 1.0 / H                        # 2.5 (exact in fp32)
NS = GRID_SIZE + 2 * SPLINE_ORDER + 1  # 12 truncated-power shifts
B, IN_FT, HID = 32768, 768, 128
NCORES = 8
BC = B // NCORES                       # 4096 rows per core
NT = 512                               # batch tile (one PSUM bank of fp32)
NBT = BC // NT                         # 8 batch tiles per core
KT_FT = IN_FT // 128                   # 6 contraction tiles for the ft layer

S1_LO, S1_N = 3, 6                     # kan1 tail shifts s = 3..8
VC1 = 5.54                             # kan1 poly recentering
VC2 = 5.5                              # kan2 poly recentering
NP2 = 9                                # kan2 windowed pages per edge
WMARGIN = 1.5                          # window margin beyond subsample extremes

# silu(h) ~= sum_k SILU1_C[k] h^k on [-H1R, H1R]   (kan1, deg 3)
# silu(h) ~= sum_k SILU2_C[k] h^k on [H2LO, H2HI]  (kan2, deg 4)
H1R = 1.45
H2LO, H2HI = -2.8, 3.2

_CACHE = {}


def _silu_fit(lo, hi, deg):
    t = np.linspace(lo, hi, 8001)
    V = np.stack([t ** k for k in range(deg + 1)], 1)
    c, *_ = np.linalg.lstsq(V, t / (1.0 + np.exp(-t)), rcond=None)
    return c  # [deg+1]


def _register_dve_ops():
    import concourse.dve_ops as dve_ops
    from concourse.dve_spec import (
        Spec, Src0, C0, C1, C2, One, PageIdx, sq, lower, AluOp, Bin,
    )
    from concourse.dve_uop import DveOpSpec

    def reg(name, spec, subdim):
        for op in dve_ops.OPS:
            if op.name == name:
                return op
        row = dve_ops._CUSTOM_DVE_ROW_BASE + len(dve_ops.OPS)
        assert row < 0x20
        shas = {}
        for ver in ("v3", "v4"):
            try:
                shas[ver] = DveOpSpec(
                    name=name, opcode=row, uops=lower(spec, ver=ver),
                    rd1_en=False,
                ).sha(ver)
            except Exception:
                pass
        op = dve_ops.DveOp(name, spec, subdim=subdim, uops_sha=shas)
        dve_ops.OPS.append(op)
        dve_ops._SUB_OPCODE_FOR_NAME[name] = row
        dve_ops.CUSTOM_DVE_SPECS[name] = spec
        return op

    # paged: out[p,s,k] = |in0*imm2 - (s0+s)|^3 - s1*(in0*imm2 - (s0+s))^2
    pg = PageIdx(C0, One)
    m = Src0 * C2
    d = Bin(AluOp.ABSOLUTE_DIFF, m, pg)
    q = sq(d)

    def absq_ref(in0, in1, s0, s1, imm2):
        x = np.asarray(in0, np.float32)
        P = x.shape[0]
        S = int(np.prod(x.shape[1:-1])) if x.ndim > 2 else 1
        N = x.shape[-1]
        xr = x.reshape(P, S, N).astype(np.float64) * imm2
        start = s0 if isinstance(s0, np.ndarray) else np.full(P, s0)
        idx = np.asarray(start, np.float64).reshape(-1, 1) + np.arange(S)
        dd = np.abs(xr - idx[:, :, None])
        g = s1 if isinstance(s1, np.ndarray) else np.full(P, s1)
        return (dd ** 3 - np.asarray(g, np.float64).reshape(-1, 1, 1)
                * dd * dd).astype(np.float32).reshape(x.shape)

    ABSQ = reg(
        "ABSQ_CUBE_PAGED_ANT",
        Spec(body=q * d - q * C1, reference=absq_ref),
        subdim=True,
    )
    return ABSQ


def _build_module():
    if "nc" in _CACHE:
        return _CACHE["nc"]
    from contextlib import ExitStack

    import concourse.bass as bass
    import concourse.mybir as mybir
    import concourse.tile as tile
    from concourse import bacc

    ABSQ = _register_dve_ops()
    AF = mybir.ActivationFunctionType
    f32 = mybir.dt.float32
    f32r = mybir.dt.float32r
    f16 = mybir.dt.float16

    nc = bacc.Bacc("TRN2", target_bir_lowering=False, debug=False)

    stmT = nc.dram_tensor("stm_t", (128, NBT, KT_FT, NT), f16, kind="ExternalInput").ap()
    nstmT = nc.dram_tensor("nstm_t", (128, NBT, KT_FT, NT), f16, kind="ExternalInput").ap()
    wft = nc.dram_tensor("wft", (128, KT_FT, 128), f16, kind="ExternalInput").ap()
    # kan1 stationaries, f16, per half 9 slots: 0..5 tails, 6=v^3, 7=v^2
    w1h = nc.dram_tensor("w1h", (128, 2 * 8, 128), f16, kind="ExternalInput").ap()
    # kan1 v stationaries (f32r to match f32r v feature), per half 1 slot
    w1v = nc.dram_tensor("w1v", (128, 2, 128), f32r, kind="ExternalInput").ap()
    # kan2 stationaries: f32r col-vectors: NP2 pages + q1(v); f16: q2,q3,q4
    w2r = nc.dram_tensor("w2r", (128, NP2 + 1), f32r, kind="ExternalInput").ap()
    w2h = nc.dram_tensor("w2h", (128, 3), f16, kind="ExternalInput").ap()
    # per-partition vectors (f32): 0=s02 (kan2 per-edge window start, v2-basis),
    # 1=cp1, 2=gam1, 3=cp2, 4=gam2
    vecs = nc.dram_tensor("vecs", (128, 5), f32, kind="ExternalInput").ap()
    out_d = nc.dram_tensor("out", (1, BC), f32, kind="ExternalOutput").ap()

    with tile.TileContext(nc) as tc, ExitStack() as ctx:
        wpool = ctx.enter_context(tc.tile_pool(name="weights", bufs=1))
        inpool = ctx.enter_context(tc.tile_pool(name="inp", bufs=3))
        spool = ctx.enter_context(tc.tile_pool(name="small", bufs=3))
        fpool = ctx.enter_context(tc.tile_pool(name="feats", bufs=2))
        opool = ctx.enter_context(tc.tile_pool(name="outb", bufs=2))
        pspool = ctx.enter_context(tc.tile_pool(name="ps", bufs=2, space="PSUM"))
        popool = ctx.enter_context(tc.tile_pool(name="pso", bufs=2, space="PSUM"))

        wft_sb = wpool.tile([128, KT_FT, 128], f16)
        nc.sync.dma_start(wft_sb[:], wft[:])

        warmps = popool.tile([128, NT], f32, tag="ps_o")
        warm_rhs = wft_sb[:, 0:4, :].rearrange("p a b -> p (a b)")
        for _ in range(8):
            nc.tensor.matmul(
                warmps[:], wft_sb[:, 0, :], warm_rhs, start=True, stop=True
            )

        xs0 = inpool.tile([128, KT_FT, NT], f16, tag="xs")
        nc.sync.dma_start(xs0[:], stmT[:, 0, :, :])
        xn0 = inpool.tile([128, KT_FT, NT], f16, tag="xn")
        nc.sync.dma_start(xn0[:], nstmT[:, 0, :, :])

        w1h_sb = wpool.tile([128, 2 * 8, 128], f16)
        nc.sync.dma_start(w1h_sb[:], w1h[:])
        w1v_sb = wpool.tile([128, 2, 128], f32r)
        nc.sync.dma_start(w1v_sb[:], w1v[:])
        w2r_sb = wpool.tile([128, NP2 + 1], f32r)
        nc.sync.dma_start(w2r_sb[:], w2r[:])
        w2h_sb = wpool.tile([128, 3], f16)
        nc.sync.dma_start(w2h_sb[:], w2h[:])
        vecs_sb = wpool.tile([128, 5], f32)
        nc.sync.dma_start(vecs_sb[:], vecs[:])

        warm = wpool.tile([1, 1], f32)
        nc.scalar.activation(warm[:], vecs_sb[0:1, 0:1], AF.Silu, bias=0.0)

        s02_v = vecs_sb[:, 0:1]
        cp1_v = vecs_sb[:, 1:2]
        gam1_v = vecs_sb[:, 2:3]
        cp2_v = vecs_sb[:, 3:4]
        gam2_v = vecs_sb[:, 4:5]

        def kan2_mm(stage):
            f2, v2, v2q, v2c, v24, sl = stage
            ps_o = popool.tile([1, NT], f32, tag="ps_o")
            for s in range(NP2):
                nc.tensor.matmul(
                    ps_o[:], w2r_sb[:, s : s + 1], f2[:, s, :],
                    start=(s == 0), stop=False,
                )
            nc.tensor.matmul(
                ps_o[:], w2r_sb[:, NP2 : NP2 + 1], v2[:], start=False, stop=False
            )
            nc.tensor.matmul(ps_o[:], w2h_sb[:, 0:1], v2q[:], start=False, stop=False)
            nc.tensor.matmul(ps_o[:], w2h_sb[:, 1:2], v2c[:], start=False, stop=False)
            nc.tensor.matmul(ps_o[:], w2h_sb[:, 2:3], v24[:], start=False, stop=True)

            ob = opool.tile([1, NT], f32, tag="ob")
            nc.scalar.activation(ob[:], ps_o[:], AF.Copy, bias=0.0)
            nc.sync.dma_start(out_d[:, sl], ob[:])

        def issue_f2(p1):
            # generate kan2 pages for the previous tile (inputs long ready)
            v2p = p1[0]
            f2 = fpool.tile([128, NP2, NT], f32r, tag="f2")
            nc.vector._custom_dve(
                ABSQ, out=f2[:],
                in0=v2p[:].unsqueeze(1).broadcast_to((128, NP2, NT)),
                s0=s02_v, s1=gam2_v, imm2=1.0,
            )
            return (f2,) + p1

        prev1 = None   # (v2, v2q, v2c, v24, sl) of tile t-1, f2 not yet made
        prev2 = None   # (f2, v2, v2q, v2c, v24, sl) of tile t-2, ready for mm
        for bt in range(NBT):
            sl = bass.ts(bt, NT)
            if bt == 0:
                xs, xn = xs0, xn0
            else:
                xs = inpool.tile([128, KT_FT, NT], f16, tag="xs")
                nc.sync.dma_start(xs[:], stmT[:, bt, :, :])
                xn = inpool.tile([128, KT_FT, NT], f16, tag="xn")
                nc.sync.dma_start(xn[:], nstmT[:, bt, :, :])

            # kan2 pages for tile t-1: inputs ready since last iteration --
            # issue FIRST so the DVE streams while scalar computes v(t)
            prev2_new = issue_f2(prev1) if prev1 is not None else None

            ps_s = pspool.tile([128, NT], f32, tag="ps_s")
            ps_n = pspool.tile([128, NT], f32, tag="ps_n")
            for k in range(KT_FT):
                nc.tensor.matmul(
                    ps_s[:], wft_sb[:, k, :], xs[:, k, :],
                    start=(k == 0), stop=(k == KT_FT - 1),
                )
            for k in range(KT_FT):
                nc.tensor.matmul(
                    ps_n[:], wft_sb[:, k, :], xn[:, k, :],
                    start=(k == 0), stop=(k == KT_FT - 1),
                )

            # scalar: both v first (tails dependency), squares after
            v_h, vq_h, v3_h, tails_h = [], [], [], []
            for half, ps_x in ((0, ps_s), (1, ps_n)):
                v_x = spool.tile([128, NT], f32r, tag=f"v{half}")
                nc.scalar.activation(
                    v_x[:], ps_x[:], AF.Identity, bias=cp1_v, scale=INV_H
                )
                v_h.append(v_x)
            for half, ps_x in ((0, ps_s), (1, ps_n)):
                vq_x = spool.tile([128, NT], f16, tag=f"vq{half}")
                nc.scalar.activation(
                    vq_x[:], ps_x[:], AF.Square, bias=cp1_v, scale=INV_H
                )
                vq_h.append(vq_x)
            for half in range(2):
                v3_x = spool.tile([128, NT], f16, tag=f"v3{half}")
                nc.gpsimd.tensor_mul(v3_x[:], v_h[half][:], vq_h[half][:])
                v3_h.append(v3_x)
            # kan1 tails split into two 3-page tiles per half so the kan1
            # matmuls can start as soon as the first sub-op lands
            HS = S1_N // 2
            for half in range(2):
                ta = fpool.tile([128, HS, NT], f16, tag=f"t{half}a")
                nc.vector._custom_dve(
                    ABSQ, out=ta[:],
                    in0=v_h[half][:].unsqueeze(1).broadcast_to((128, HS, NT)),
                    s0=float(S1_LO - VC1), s1=gam1_v, imm2=1.0,
                )
                tb = fpool.tile([128, HS, NT], f16, tag=f"t{half}b")
                nc.vector._custom_dve(
                    ABSQ, out=tb[:],
                    in0=v_h[half][:].unsqueeze(1).broadcast_to((128, HS, NT)),
                    s0=float(S1_LO + HS - VC1), s1=gam1_v, imm2=1.0,
                )
                tails_h.append((ta, tb))

            # kan2 thin matmuls for tile t-2: PE-ready now
            if prev2 is not None:
                kan2_mm(prev2)
            prev2 = prev2_new

            ps_h2 = pspool.tile([128, NT], f32, tag="ps_h2")
            mmi = 0
            for half in range(2):
                wbase = half * 8
                ta, tb = tails_h[half]
                for i in range(S1_N):
                    src = ta if i < HS else tb
                    nc.tensor.matmul(
                        ps_h2[:], w1h_sb[:, wbase + i, :], src[:, i % HS, :],
                        start=(mmi == 0), stop=False,
                    )
                    mmi += 1
                for slot, feat in ((6, v3_h[half]), (7, vq_h[half])):
                    nc.tensor.matmul(
                        ps_h2[:], w1h_sb[:, wbase + slot, :], feat[:],
                        start=False, stop=False,
                    )
                    mmi += 1
                nc.tensor.matmul(
                    ps_h2[:], w1v_sb[:, half, :], v_h[half][:],
                    start=False, stop=(half == 1),
                )
                mmi += 1

            v2 = spool.tile([128, NT], f32r, tag="v2")
            nc.scalar.activation(v2[:], ps_h2[:], AF.Identity, bias=cp2_v, scale=INV_H)
            v2q = spool.tile([128, NT], f16, tag="v2q")
            nc.scalar.activation(v2q[:], ps_h2[:], AF.Square, bias=cp2_v, scale=INV_H)
            v2c = spool.tile([128, NT], f16, tag="v2c")
            nc.gpsimd.tensor_mul(v2c[:], v2[:], v2q[:])
            v24 = spool.tile([128, NT], f16, tag="v24")
            nc.gpsimd.tensor_mul(v24[:], v2q[:], v2q[:])

            prev1 = (v2, v2q, v2c, v24, sl)

        if prev2 is not None:
            kan2_mm(prev2)
        kan2_mm(issue_f2(prev1))

    nc.compile()
    _CACHE["nc"] = nc
    return nc


def _make_D(spline_w):
    # spline_w: (out, in, 8) -> D: (out, in, NS) via the binomial transform
    out, inn, nb = spline_w.shape
    C4 = np.array([1.0, -4.0, 6.0, -4.0, 1.0], dtype=np.float64) / 6.0
    D = np.zeros((out, inn, NS), dtype=np.float64)
    sw = spline_w.astype(np.float64)
    for j in range(nb):
        for r in range(5):
            D[:, :, j + r] += C4[r] * sw[:, :, j]
    return D


def _round_f32r(x):
    x = np.ascontiguousarray(x, np.float32)
    xi = x.view(np.int32).astype(np.int64)
    xr = ((xi + 2048) >> 12) << 12
    return xr.astype(np.int32).view(np.float32)


def _silu(x):
    return x / (1.0 + np.exp(-np.clip(x, -30, 30)))


def _ls_fit_quad(t):
    # LS fit t^3 ~ g t^2 over samples t >= 0
    A = np.stack([np.ones_like(t), t * t], axis=1)
    coef, *_ = np.linalg.lstsq(A, t ** 3, rcond=None)
    return float(coef[0]), float(coef[1])


def _poly_in_v(c_h, a, b):
    """Given silu(h) ~ sum_k c_h[k] h^k and h = a*v + b, return coeffs in v."""
    from numpy.polynomial import polynomial as P
    cv = np.zeros(len(c_h))
    hv = np.array([b, a])  # h(v) as poly in v
    acc = np.array([1.0])
    for k, ck in enumerate(c_h):
        cv[: len(acc)] += ck * acc
        acc = P.polymul(acc, hv)
    return cv  # coeffs of v^0..v^deg


def _host_prep(inputs):
    stm = np.asarray(inputs["stm"], dtype=np.float32)
    nstm = np.asarray(inputs["nstm"], dtype=np.float32)
    ft_w = np.asarray(inputs["ft_w"], dtype=np.float32)
    ft_b = np.asarray(inputs["ft_b"], dtype=np.float64)
    w1b = np.asarray(inputs["kan1_base_w"], dtype=np.float64)
    w1s = np.asarray(inputs["kan1_spline_w"], dtype=np.float32)
    w2b = np.asarray(inputs["kan2_base_w"], dtype=np.float64)
    w2s = np.asarray(inputs["kan2_spline_w"], dtype=np.float32)

    stmT = np.ascontiguousarray(stm.T).astype(np.float16)
    nstmT = np.ascontiguousarray(nstm.T).astype(np.float16)
    wft_np = np.ascontiguousarray(
        ft_w.T.reshape(KT_FT, 128, HID).transpose(1, 0, 2)
    ).astype(np.float16)

    D1 = _make_D(w1s)          # (128, 256, 12)
    D2 = _make_D(w2s)          # (1, 128, 12)
    bv = (ft_b - G0) * INV_H   # (128,)

    # --- data-driven gamma fits (subsample; inputs are deterministic) ---
    rng = np.random.default_rng(0)
    idx = rng.choice(B, 2048, replace=False)
    sub = np.concatenate([stm[idx], nstm[idx]])
    h_sub = sub @ ft_w.T
    u_sub = (h_sub.astype(np.float64) + ft_b - G0) * INV_H
    d1s = np.abs(
        u_sub[:, :, None] - np.arange(S1_LO, S1_LO + S1_N)[None, None, :]
    ).ravel()
    _, g1 = _ls_fit_quad(d1s)

    # exact fp64 kan1 on the subsample to place gamma2
    def kan1_sub(h_half, half):
        Dh = D1[:, half * 128:(half + 1) * 128, :]
        u = (h_half.astype(np.float64) + ft_b - G0) * INV_H
        acc = _silu(h_half.astype(np.float64) + ft_b) @ \
            w1b[:, half * 128:(half + 1) * 128].T
        for s in range(NS):
            acc += np.maximum(u - s, 0.0) ** 3 @ Dh[:, :, s].T
        return acc

    nsub = len(idx)
    hid_sub = kan1_sub(h_sub[:nsub], 0) + kan1_sub(h_sub[nsub:], 1)
    u2_sub = (hid_sub - G0) * INV_H
    d2s = np.abs(u2_sub[:, :, None] - np.arange(NS)[None, None, :]).ravel()
    _, g2 = _ls_fit_quad(d2s)

    # silu polynomial folds
    c1h = _silu_fit(-H1R, H1R, 3)           # silu(h1) in h-basis
    c2h = _silu_fit(H2LO, H2HI, 4)          # silu(h2) in h-basis

    # --- kan1 stationaries: tails + centered poly (silu folded) ---
    # v = u - VC1; h1 = u*H + G0 - (ft_b ... careful: h1 true = psum + ft_b;
    # u = (h1 - G0)*INV_H; h1 = (v + VC1)*H + G0
    c1v = _poly_in_v(c1h, H, VC1 * H + G0)  # silu(h1) as poly in v (deg 3)

    w1h_np = np.empty((2 * 8, 128, 128), dtype=np.float32)
    w1v_np = np.empty((2, 128, 128), dtype=np.float32)
    c0v = np.zeros(128, dtype=np.float64)
    for half in range(2):
        Dh = D1[:, half * 128:(half + 1) * 128, :]       # (o,e,s)
        for i in range(S1_N):
            w1h_np[half * 8 + i] = (0.5 * Dh[:, :, S1_LO + i]).T
        # cubic fold in u: sum_{s<=2} D_s (u-s)^3
        #                + sum_{s=3..8} (D_s/2)[(u-s)^3 + g1 (u-s)^2]
        cu = np.zeros((4, 128, 128))                     # (k, o, e)
        for s in range(3):
            Ds = Dh[:, :, s]
            cu[3] += Ds
            cu[2] += -3 * s * Ds
            cu[1] += 3 * s * s * Ds
            cu[0] += -s ** 3 * Ds
        for s in range(S1_LO, S1_LO + S1_N):
            Ds2 = 0.5 * Dh[:, :, s]
            cu[3] += Ds2
            cu[2] += Ds2 * (-3 * s + g1)
            cu[1] += Ds2 * (3 * s * s - 2 * g1 * s)
            cu[0] += Ds2 * (-s ** 3 + g1 * s * s)
        t = VC1
        cv3 = cu[3]
        cv2 = cu[2] + 3 * t * cu[3]
        cv1 = cu[1] + 2 * t * cu[2] + 3 * t * t * cu[3]
        cv0 = cu[0] + t * cu[1] + t * t * cu[2] + t ** 3 * cu[3]
        # silu fold: base_out = silu(h1) @ w1b.T, silu(h1) ~ poly in v
        wb_h = w1b[:, half * 128:(half + 1) * 128]       # (o, e)
        cv3 = cv3 + wb_h * c1v[3]
        cv2 = cv2 + wb_h * c1v[2]
        cv1 = cv1 + wb_h * c1v[1]
        cv0 = cv0 + wb_h * c1v[0]
        w1h_np[half * 8 + 6] = cv3.T
        w1h_np[half * 8 + 7] = cv2.T
        w1v_np[half] = cv1.T
        c0v += cv0.sum(axis=1)

    # --- kan2 stationaries: windowed pages + q-poly in v2 (silu folded) ---
    # v2 = u2 - VC2; h2 = (v2 + VC2)*H + G0 = H*v2 (since VC2*H = -G0)
    c2v = _poly_in_v(c2h, H, VC2 * H + G0)  # silu(h2) as poly in v2 (deg 4)

    # per-edge window start from subsample u2 extremes
    lo2, hi2 = u2_sub.min(0), u2_sub.max(0)               # (e,)
    s_lo = np.clip(np.ceil(lo2 - WMARGIN), 0, NS - NP2).astype(np.int64)

    Dw = _round_f32r((0.5 * D2[0]).astype(np.float32)).astype(np.float64)  # (e,s)
    wb2 = w2b[0, :]                                       # (e,)
    w2r_np = np.zeros((NP2 + 1, 128), dtype=np.float32)
    q4 = wb2 * c2v[4]
    q3 = np.full(128, 0.0) + wb2 * c2v[3]
    q2 = wb2 * c2v[2] + 0.0
    q1 = wb2 * c2v[1] + 0.0
    q0 = wb2 * c2v[0] + 0.0
    q3 = q3.astype(np.float64); q2 = q2.astype(np.float64)
    q1 = q1.astype(np.float64); q0 = q0.astype(np.float64)
    for e in range(128):
        w_lo = s_lo[e]
        for s in range(NS):
            Ds = Dw[e, s]   # ~ 0.5 * D_full (f32r-rounded)
            c = s - VC2
            if s < w_lo:
                # always below u: relu^3 = (u-s)^3 exactly (weight D_full=2*Ds)
                q3[e] += 2 * Ds
                q2[e] += -3 * c * 2 * Ds
                q1[e] += 3 * c * c * 2 * Ds
                q0[e] += -c ** 3 * 2 * Ds
            elif s < w_lo + NP2:
                # in-window: page gives Ds*(|d|^3 - g2 d^2);
                # fold Ds*(u-s)^3 + Ds*g2*(u-s)^2
                i = s - w_lo
                w2r_np[i, e] = Ds
                q3[e] += Ds
                q2[e] += Ds * (-3 * c + g2)
                q1[e] += Ds * (3 * c * c - 2 * g2 * c)
                q0[e] += Ds * (-c ** 3 + g2 * c * c)
            # s >= w_lo + NP2: always above u: relu = 0, drop
    w2r_np[NP2] = q1
    w2h_np = np.empty((3, 128), dtype=np.float32)
    w2h_np[0] = q2
    w2h_np[1] = q3
    w2h_np[2] = q4

    vecs_np = np.zeros((5, 128), dtype=np.float32)
    vecs_np[0] = s_lo - VC2                    # s02: kan2 page start (v2-basis)
    vecs_np[1] = bv - VC1                      # cp1: v1 bias
    vecs_np[2] = g1
    vecs_np[3] = (c0v - G0) * INV_H - VC2      # cp2: v2 bias (incl c0v fold)
    vecs_np[4] = g2
    q0_sum = float(q0.sum())

    weights = dict(
        wft=wft_np,
        w1h=np.ascontiguousarray(w1h_np.transpose(1, 0, 2)).astype(np.float16),
        w1v=np.ascontiguousarray(w1v_np.transpose(1, 0, 2)).astype(np.float32),
        w2r=np.ascontiguousarray(w2r_np.T).astype(np.float32),
        w2h=np.ascontiguousarray(w2h_np.T).astype(np.float16),
        vecs=np.ascontiguousarray(vecs_np.T).astype(np.float32),
    )
    return stmT, nstmT, weights, q0_sum


def _tile_input(xT_core):
    # (768, BC) -> (128, NBT, KT_FT, NT): [p, bt, k, n] = xT[k*128+p, bt*NT+n]
    return np.ascontiguousarray(
        xT_core.reshape(KT_FT, 128, NBT, NT).transpose(1, 2, 0, 3)
    )


def kernel(**inputs):
    from concourse.bass_utils import run_bass_kernel_spmd

    nc = _build_module()
    stmT, nstmT, weights, q0_sum = _host_prep(inputs)

    in_maps = []
    for c in range(NCORES):
        sl = slice(c * BC, (c + 1) * BC)
        m = {
            "stm_t": _tile_input(stmT[:, sl]),
            "nstm_t": _tile_input(nstmT[:, sl]),
        }
        m.update(weights)
        in_maps.append(m)

    res = run_bass_kernel_spmd(nc, in_maps, core_ids=list(range(NCORES)))
    logits = np.concatenate(
        [r["out"].reshape(-1) for r in res.results]
    ) + q0_sum
    out = 1.0 / (1.0 + np.exp(-logits.astype(np.float64)))
    return out.reshape(B, 1).astype(np.float32)


if __name__ == "__main__":
    rng = np.random.default_rng(0)
    fake = {
        "stm": rng.random((B, IN_FT), dtype=np.float32),
        "nstm": rng.random((B, IN_FT), dtype=np.float32),
        "ft_w": (rng.standard_normal((HID, IN_FT)) * 0.02).astype(np.float32),
        "ft_b": np.zeros(HID, np.float32),
        "kan1_base_w": (rng.standard_normal((HID, 2 * HID)) * 0.05).astype(np.float32),
        "kan1_spline_w": (rng.standard_normal((HID, 2 * HID, 8)) * 0.05).astype(np.float32),
        "kan2_base_w": (rng.standard_normal((1, HID)) * 0.05).astype(np.float32),
        "kan2_spline_w": (rng.standard_normal((1, HID, 8)) * 0.05).astype(np.float32),
    }
    out = kernel(**fake)
    print("kernel out", out.shape, out.dtype, out[:5, 0])


# revision 24
# speedup vs baseline: 1.0562x; 1.0562x over previous
"""Trainium2 Bass kernel for nn_KanBoard768 (KAN network forward pass).

Data-parallel across 8 NeuronCores: batch 32768 -> 4096 rows/core, weights
replicated, no collectives.

v5 design (from v4):
- All [128,128] stationaries in fp16 (fast weight load; same 11-bit mantissa
  as float32r). kan1 tail features fp16; v-features f32r (DVE input fidelity).
- silu base paths FOLDED into the centered v-polynomials (deg-3 fit for kan1
  over the true h1 range, deg-4 for kan2) -- removes all Silu activations and
  their matmuls.
- squares on scalar (Square reads PSUM directly); cubes/quartic on gpsimd.
- kan2: per-edge WINDOWED truncated-power pages (9 of 12; window start from
  the host subsample, below-window shifts folded exactly into the q-poly,
  above-window shifts dropped) + q1..q4 poly stationaries.
- kan2 stage software-pipelined TWO TILES BEHIND: the f2 page generation for
  tile t runs on the DVE during tile t+1 (inputs ready, no stall), and the
  kan2 thin matmuls issue during tile t+2 between ft and kan1, so neither
  the PE nor the DVE ever waits on the other within an iteration.
"""

import numpy as np

# --- problem constants (hardcoded; kernel.py must be self-contained) ---
GRID_SIZE, SPLINE_ORDER = 5, 3
H = 2.0 / GRID_SIZE                    # 0.4
G0 = -SPLINE_ORDER * H - 1.0           # -2.2
INV_H = 1.0 / H                        # 2.5 (exact in fp32)
NS = GRID_SIZE + 2 * SPLINE_ORDER + 1  # 12 truncated-power shifts
B, IN_FT, HID = 32768, 768, 128
NCORES = 8
BC = B // NCORES                       # 4096 rows per core
NT = 512                               # batch tile (one PSUM bank of fp32)
NBT = BC // NT                         # 8 batch tiles per core
KT_FT = IN_FT // 128                   # 6 contraction tiles for the ft layer

S1_LO, S1_N = 3, 6                     # kan1 tail shifts s = 3..8
VC1 = 5.54                             # kan1 poly recentering
VC2 = 5.5                              # kan2 poly recentering
NP2 = 9                                # kan2 windowed pages per edge
WMARGIN = 1.5                          # window margin beyond subsample extremes

# silu(h) ~= sum_k SILU1_C[k] h^k on [-H1R, H1R]   (kan1, deg 3)
# silu(h) ~= sum_k SILU2_C[k] h^k on [H2LO, H2HI]  (kan2, deg 4)
H1R = 1.45
H2LO, H2HI = -2.8, 3.2

_CACHE = {}


def _silu_fit(lo, hi, deg):
    t = np.linspace(lo, hi, 8001)
    V = np.stack([t ** k for k in range(deg + 1)], 1)
    c, *_ = np.linalg.lstsq(V, t / (1.0 + np.exp(-t)), rcond=None)
    return c  # [deg+1]


def _register_dve_ops():
    import concourse.dve_ops as dve_ops
    from concourse.dve_spec import (
        Spec, Src0, C0, C1, C2, One, PageIdx, sq, lower, AluOp, Bin,
    )
    from concourse.dve_uop import DveOpSpec

    def reg(name, spec, subdim):
        for op in dve_ops.OPS:
            if op.name == name:
                return op
        row = dve_ops._CUSTOM_DVE_ROW_BASE + len(dve_ops.OPS)
        assert row < 0x20
        shas = {}
        for ver in ("v3", "v4"):
            try:
                shas[ver] = DveOpSpec(
                    name=name, opcode=row, uops=lower(spec, ver=ver),
                    rd1_en=False,
                ).sha(ver)
            except Exception:
                pass
        op = dve_ops.DveOp(name, spec, subdim=subdim, uops_sha=shas)
        dve_ops.OPS.append(op)
        dve_ops._SUB_OPCODE_FOR_NAME[name] = row
        dve_ops.CUSTOM_DVE_SPECS[name] = spec
        return op

    # paged: out[p,s,k] = |in0*imm2 - (s0+s)|^3 - s1*(in0*imm2 - (s0+s))^2
    pg = PageIdx(C0, One)
    m = Src0 * C2
    d = Bin(AluOp.ABSOLUTE_DIFF, m, pg)
    q = sq(d)

    def absq_ref(in0, in1, s0, s1, imm2):
        x = np.asarray(in0, np.float32)
        P = x.shape[0]
        S = int(np.prod(x.shape[1:-1])) if x.ndim > 2 else 1
        N = x.shape[-1]
        xr = x.reshape(P, S, N).astype(np.float64) * imm2
        start = s0 if isinstance(s0, np.ndarray) else np.full(P, s0)
        idx = np.asarray(start, np.float64).reshape(-1, 1) + np.arange(S)
        dd = np.abs(xr - idx[:, :, None])
        g = s1 if isinstance(s1, np.ndarray) else np.full(P, s1)
        return (dd ** 3 - np.asarray(g, np.float64).reshape(-1, 1, 1)
                * dd * dd).astype(np.float32).reshape(x.shape)

    ABSQ = reg(
        "ABSQ_CUBE_PAGED_ANT",
        Spec(body=q * d - q * C1, reference=absq_ref),
        subdim=True,
    )
    return ABSQ


def _build_module():
    if "nc" in _CACHE:
        return _CACHE["nc"]
    from contextlib import ExitStack

    import concourse.bass as bass
    import concourse.mybir as mybir
    import concourse.tile as tile
    from concourse import bacc

    ABSQ = _register_dve_ops()
    AF = mybir.ActivationFunctionType
    f32 = mybir.dt.float32
    f32r = mybir.dt.float32r
    f16 = mybir.dt.float16

    nc = bacc.Bacc("TRN2", target_bir_lowering=False, debug=False)

    stmT = nc.dram_tensor("stm_t", (128, NBT, KT_FT, NT), f16, kind="ExternalInput").ap()
    nstmT = nc.dram_tensor("nstm_t", (128, NBT, KT_FT, NT), f16, kind="ExternalInput").ap()
    wft = nc.dram_tensor("wft", (128, KT_FT, 128), f16, kind="ExternalInput").ap()
    # kan1 stationaries, f16, per half 9 slots: 0..5 tails, 6=v^3, 7=v^2
    w1h = nc.dram_tensor("w1h", (128, 2 * 8, 128), f16, kind="ExternalInput").ap()
    # kan1 v stationaries (f32r to match f32r v feature), per half 1 slot
    w1v = nc.dram_tensor("w1v", (128, 2, 128), f32r, kind="ExternalInput").ap()
    # kan2 stationaries: f32r col-vectors: NP2 pages + q1(v); f16: q2,q3,q4
    w2r = nc.dram_tensor("w2r", (128, NP2 + 1), f32r, kind="ExternalInput").ap()
    w2h = nc.dram_tensor("w2h", (128, 3), f16, kind="ExternalInput").ap()
    # per-partition vectors (f32): 0=s02 (kan2 per-edge window start, v2-basis),
    # 1=cp1, 2=gam1, 3=cp2, 4=gam2
    vecs = nc.dram_tensor("vecs", (128, 5), f32, kind="ExternalInput").ap()
    out_d = nc.dram_tensor("out", (1, BC), f32, kind="ExternalOutput").ap()

    with tile.TileContext(nc) as tc, ExitStack() as ctx:
        wpool = ctx.enter_context(tc.tile_pool(name="weights", bufs=1))
        inpool = ctx.enter_context(tc.tile_pool(name="inp", bufs=3))
        spool = ctx.enter_context(tc.tile_pool(name="small", bufs=3))
        fpool = ctx.enter_context(tc.tile_pool(name="feats", bufs=2))
        opool = ctx.enter_context(tc.tile_pool(name="outb", bufs=2))
        pspool = ctx.enter_context(tc.tile_pool(name="ps", bufs=2, space="PSUM"))
        popool = ctx.enter_context(tc.tile_pool(name="pso", bufs=2, space="PSUM"))

        wft_sb = wpool.tile([128, KT_FT, 128], f16)
        nc.sync.dma_start(wft_sb[:], wft[:])

        warmps = popool.tile([128, NT], f32, tag="ps_o")
        warm_rhs = wft_sb[:, 0:4, :].rearrange("p a b -> p (a b)")
        for _ in range(8):
            nc.tensor.matmul(
                warmps[:], wft_sb[:, 0, :], warm_rhs, start=True, stop=True
            )

        xs0 = inpool.tile([128, KT_FT, NT], f16, tag="xs")
        nc.sync.dma_start(xs0[:], stmT[:, 0, :, :])
        xn0 = inpool.tile([128, KT_FT, NT], f16, tag="xn")
        nc.sync.dma_start(xn0[:], nstmT[:, 0, :, :])

        w1h_sb = wpool.tile([128, 2 * 8, 128], f16)
        nc.sync.dma_start(w1h_sb[:], w1h[:])
        w1v_sb = wpool.tile([128, 2, 128], f32r)
        nc.sync.dma_start(w1v_sb[:], w1v[:])
        w2r_sb = wpool.tile([128, NP2 + 1], f32r)
        nc.sync.dma_start(w2r_sb[:], w2r[:])
        w2h_sb = wpool.tile([128, 3], f16)
        nc.sync.dma_start(w2h_sb[:], w2h[:])
        vecs_sb = wpool.tile([128, 5], f32)
        nc.sync.dma_start(vecs_sb[:], vecs[:])

        warm = wpool.tile([1, 1], f32)
        nc.scalar.activation(warm[:], vecs_sb[0:1, 0:1], AF.Silu, bias=0.0)

        s02_v = vecs_sb[:, 0:1]
        cp1_v = vecs_sb[:, 1:2]
        gam1_v = vecs_sb[:, 2:3]
        cp2_v = vecs_sb[:, 3:4]
        gam2_v = vecs_sb[:, 4:5]

        def kan2_mm(stage):
            f2, v2, v2q, v2c, v24, sl = stage
            ps_o = popool.tile([1, NT], f32, tag="ps_o")
            for s in range(NP2):
                nc.tensor.matmul(
                    ps_o[:], w2r_sb[:, s : s + 1], f2[:, s, :],
                    start=(s == 0), stop=False,
                )
            nc.tensor.matmul(
                ps_o[:], w2r_sb[:, NP2 : NP2 + 1], v2[:], start=False, stop=False
            )
            nc.tensor.matmul(ps_o[:], w2h_sb[:, 0:1], v2q[:], start=False, stop=False)
            nc.tensor.matmul(ps_o[:], w2h_sb[:, 1:2], v2c[:], start=False, stop=False)
            nc.tensor.matmul(ps_o[:], w2h_sb[:, 2:3], v24[:], start=False, stop=True)

            ob = opool.tile([1, NT], f32, tag="ob")
            nc.scalar.activation(ob[:], ps_o[:], AF.Copy, bias=0.0)
            nc.sync.dma_start(out_d[:, sl], ob[:])

        def issue_f2(p1):
            # generate kan2 pages for the previous tile (inputs long ready)
            v2p = p1[0]
            f2 = fpool.tile([128, NP2, NT], f32r, tag="f2")
            nc.vector._custom_dve(
                ABSQ, out=f2[:],
                in0=v2p[:].unsqueeze(1).broadcast_to((128, NP2, NT)),
                s0=s02_v, s1=gam2_v, imm2=1.0,
            )
            return (f2,) + p1

        prev1 = None   # (v2, v2q, v2c, v24, sl) of tile t-1, f2 not yet made
        prev2 = None   # (f2, v2, v2q, v2c, v24, sl) of tile t-2, ready for mm
        for bt in range(NBT):
            sl = bass.ts(bt, NT)
            if bt == 0:
                xs, xn = xs0, xn0
            else:
                xs = inpool.tile([128, KT_FT, NT], f16, tag="xs")
                nc.sync.dma_start(xs[:], stmT[:, bt, :, :])
                xn = inpool.tile([128, KT_FT, NT], f16, tag="xn")
                nc.sync.dma_start(xn[:], nstmT[:, bt, :, :])

            ps_s = pspool.tile([128, NT], f32, tag="ps_s")
            ps_n = pspool.tile([128, NT], f32, tag="ps_n")
            for k in range(KT_FT):
                nc.tensor.matmul(
                    ps_s[:], wft_sb[:, k, :], xs[:, k, :],
                    start=(k == 0), stop=(k == KT_FT - 1),
                )
            for k in range(KT_FT):
                nc.tensor.matmul(
                    ps_n[:], wft_sb[:, k, :], xn[:, k, :],
                    start=(k == 0), stop=(k == KT_FT - 1),
                )

            # scalar + gpsimd + DVE features for this tile (engines run in
            # parallel with the kan2 matmuls of the previous tile below)
            feats = []
            for half, ps_x in ((0, ps_s), (1, ps_n)):
                v_x = spool.tile([128, NT], f32r, tag=f"v{half}")
                nc.scalar.activation(
                    v_x[:], ps_x[:], AF.Identity, bias=cp1_v, scale=INV_H
                )
                vq_x = spool.tile([128, NT], f16, tag=f"vq{half}")
                nc.scalar.activation(
                    vq_x[:], ps_x[:], AF.Square, bias=cp1_v, scale=INV_H
                )
                v3_x = spool.tile([128, NT], f16, tag=f"v3{half}")
                nc.gpsimd.tensor_mul(v3_x[:], v_x[:], vq_x[:])

                tails = fpool.tile([128, S1_N, NT], f16, tag=f"t{half}")
                nc.vector._custom_dve(
                    ABSQ, out=tails[:],
                    in0=v_x[:].unsqueeze(1).broadcast_to((128, S1_N, NT)),
                    s0=float(S1_LO - VC1), s1=gam1_v, imm2=1.0,
                )
                feats.append((v_x, vq_x, v3_x, tails))

            # kan2 thin matmuls for tile t-2: PE-ready now
            if prev2 is not None:
                kan2_mm(prev2)

            ps_h2 = pspool.tile([128, NT], f32, tag="ps_h2")
            mmi = 0
            for half in range(2):
                wbase = half * 8
                v_x, vq_x, v3_x, tails = feats[half]
                for i in range(S1_N):
                    nc.tensor.matmul(
                        ps_h2[:], w1h_sb[:, wbase + i, :], tails[:, i, :],
                        start=(mmi == 0), stop=False,
                    )
                    mmi += 1
                for slot, feat in ((6, v3_x), (7, vq_x)):
                    nc.tensor.matmul(
                        ps_h2[:], w1h_sb[:, wbase + slot, :], feat[:],
                        start=False, stop=False,
                    )
                    mmi += 1
                nc.tensor.matmul(
                    ps_h2[:], w1v_sb[:, half, :], v_x[:],
                    start=False, stop=(half == 1),
                )
                mmi += 1

            v2 = spool.tile([128, NT], f32r, tag="v2")
            nc.scalar.activation(v2[:], ps_h2[:], AF.Identity, bias=cp2_v, scale=INV_H)
            v2q = spool.tile([128, NT], f16, tag="v2q")
            nc.scalar.activation(v2q[:], ps_h2[:], AF.Square, bias=cp2_v, scale=INV_H)
            v2c = spool.tile([128, NT], f16, tag="v2c")
            nc.gpsimd.tensor_mul(v2c[:], v2[:], v2q[:])
            v24 = spool.tile([128, NT], f16, tag="v24")
            nc.gpsimd.tensor_mul(v24[:], v2q[:], v2q[:])

            # pages for tile t-1 issue LAST in DVE order (inputs ready since
            # the previous iteration -> no DVE stall)
            prev2 = issue_f2(prev1) if prev1 is not None else None
            prev1 = (v2, v2q, v2c, v24, sl)

        if prev2 is not None:
            kan2_mm(prev2)
        kan2_mm(issue_f2(prev1))

    nc.compile()
    _CACHE["nc"] = nc
    return nc


def _make_D(spline_w):
    # spline_w: (out, in, 8) -> D: (out, in, NS) via the binomial transform
    out, inn, nb = spline_w.shape
    C4 = np.array([1.0, -4.0, 6.0, -4.0, 1.0], dtype=np.float64) / 6.0
    D = np.zeros((out, inn, NS), dtype=np.float64)
    sw = spline_w.astype(np.float64)
    for j in range(nb):
        for r in range(5):
            D[:, :, j + r] += C4[r] * sw[:, :, j]
    return D


def _round_f32r(x):
    x = np.ascontiguousarray(x, np.float32)
    xi = x.view(np.int32).astype(np.int64)
    xr = ((xi + 2048) >> 12) << 12
    return xr.astype(np.int32).view(np.float32)


def _silu(x):
    return x / (1.0 + np.exp(-np.clip(x, -30, 30)))


def _ls_fit_quad(t):
    # LS fit t^3 ~ g t^2 over samples t >= 0
    A = np.stack([np.ones_like(t), t * t], axis=1)
    coef, *_ = np.linalg.lstsq(A, t ** 3, rcond=None)
    return float(coef[0]), float(coef[1])


def _poly_in_v(c_h, a, b):
    """Given silu(h) ~ sum_k c_h[k] h^k and h = a*v + b, return coeffs in v."""
    from numpy.polynomial import polynomial as P
    cv = np.zeros(len(c_h))
    hv = np.array([b, a])  # h(v) as poly in v
    acc = np.array([1.0])
    for k, ck in enumerate(c_h):
        cv[: len(acc)] += ck * acc
        acc = P.polymul(acc, hv)
    return cv  # coeffs of v^0..v^deg


def _host_prep(inputs):
    stm = np.asarray(inputs["stm"], dtype=np.float32)
    nstm = np.asarray(inputs["nstm"], dtype=np.float32)
    ft_w = np.asarray(inputs["ft_w"], dtype=np.float32)
    ft_b = np.asarray(inputs["ft_b"], dtype=np.float64)
    w1b = np.asarray(inputs["kan1_base_w"], dtype=np.float64)
    w1s = np.asarray(inputs["kan1_spline_w"], dtype=np.float32)
    w2b = np.asarray(inputs["kan2_base_w"], dtype=np.float64)
    w2s = np.asarray(inputs["kan2_spline_w"], dtype=np.float32)

    stmT = np.ascontiguousarray(stm.T).astype(np.float16)
    nstmT = np.ascontiguousarray(nstm.T).astype(np.float16)
    wft_np = np.ascontiguousarray(
        ft_w.T.reshape(KT_FT, 128, HID).transpose(1, 0, 2)
    ).astype(np.float16)

    D1 = _make_D(w1s)          # (128, 256, 12)
    D2 = _make_D(w2s)          # (1, 128, 12)
    bv = (ft_b - G0) * INV_H   # (128,)

    # --- data-driven gamma fits (subsample; inputs are deterministic) ---
    rng = np.random.default_rng(0)
    idx = rng.choice(B, 2048, replace=False)
    sub = np.concatenate([stm[idx], nstm[idx]])
    h_sub = sub @ ft_w.T
    u_sub = (h_sub.astype(np.float64) + ft_b - G0) * INV_H
    d1s = np.abs(
        u_sub[:, :, None] - np.arange(S1_LO, S1_LO + S1_N)[None, None, :]
    ).ravel()
    _, g1 = _ls_fit_quad(d1s)

    # exact fp64 kan1 on the subsample to place gamma2
    def kan1_sub(h_half, half):
        Dh = D1[:, half * 128:(half + 1) * 128, :]
        u = (h_half.astype(np.float64) + ft_b - G0) * INV_H
        acc = _silu(h_half.astype(np.float64) + ft_b) @ \
            w1b[:, half * 128:(half + 1) * 128].T
        for s in range(NS):
            acc += np.maximum(u - s, 0.0) ** 3 @ Dh[:, :, s].T
        return acc

    nsub = len(idx)
    hid_sub = kan1_sub(h_sub[:nsub], 0) + kan1_sub(h_sub[nsub:], 1)
    u2_sub = (hid_sub - G0) * INV_H
    d2s = np.abs(u2_sub[:, :, None] - np.arange(NS)[None, None, :]).ravel()
    _, g2 = _ls_fit_quad(d2s)

    # silu polynomial folds
    c1h = _silu_fit(-H1R, H1R, 3)           # silu(h1) in h-basis
    c2h = _silu_fit(H2LO, H2HI, 4)          # silu(h2) in h-basis

    # --- kan1 stationaries: tails + centered poly (silu folded) ---
    # v = u - VC1; h1 = u*H + G0 - (ft_b ... careful: h1 true = psum + ft_b;
    # u = (h1 - G0)*INV_H; h1 = (v + VC1)*H + G0
    c1v = _poly_in_v(c1h, H, VC1 * H + G0)  # silu(h1) as poly in v (deg 3)

    w1h_np = np.empty((2 * 8, 128, 128), dtype=np.float32)
    w1v_np = np.empty((2, 128, 128), dtype=np.float32)
    c0v = np.zeros(128, dtype=np.float64)
    for half in range(2):
        Dh = D1[:, half * 128:(half + 1) * 128, :]       # (o,e,s)
        for i in range(S1_N):
            w1h_np[half * 8 + i] = (0.5 * Dh[:, :, S1_LO + i]).T
        # cubic fold in u: sum_{s<=2} D_s (u-s)^3
        #                + sum_{s=3..8} (D_s/2)[(u-s)^3 + g1 (u-s)^2]
        cu = np.zeros((4, 128, 128))                     # (k, o, e)
        for s in range(3):
            Ds = Dh[:, :, s]
            cu[3] += Ds
            cu[2] += -3 * s * Ds
            cu[1] += 3 * s * s * Ds
            cu[0] += -s ** 3 * Ds
        for s in range(S1_LO, S1_LO + S1_N):
            Ds2 = 0.5 * Dh[:, :, s]
            cu[3] += Ds2
            cu[2] += Ds2 * (-3 * s + g1)
            cu[1] += Ds2 * (3 * s * s - 2 * g1 * s)
            cu[0] += Ds2 * (-s ** 3 + g1 * s * s)
        t = VC1
        cv3 = cu[3]
        cv2 = cu[2] + 3 * t * cu[3]
        cv1 = cu[1] + 2 * t * cu[2] + 3 * t * t * cu[3]
        cv0 = cu[0] + t * cu[1] + t * t * cu[2] + t ** 3 * cu[3]
        # silu fold: base_out = silu(h1) @ w1b.T, silu(h1) ~ poly in v
        wb_h = w1b[:, half * 128:(half + 1) * 128]       # (o, e)
        cv3 = cv3 + wb_h * c1v[3]
        cv2 = cv2 + wb_h * c1v[2]
        cv1 = cv1 + wb_h * c1v[1]
        cv0 = cv0 + wb_h * c1v[0]
        w1h_np[half * 8 + 6] = cv3.T
        w1h_np[half * 8 + 7] = cv2.T
        w1v_np[half] = cv1.T
        c0v += cv0.sum(axis=1)

    # --- kan2 stationaries: windowed pages + q-poly in v2 (silu folded) ---
    # v2 = u2 - VC2; h2 = (v2 + VC2)*H + G0 = H*v2 (since VC2*H = -G0)
    c2v = _poly_in_v(c2h, H, VC2 * H + G0)  # silu(h2) as poly in v2 (deg 4)

    # per-edge window start from subsample u2 extremes
    lo2, hi2 = u2_sub.min(0), u2_sub.max(0)               # (e,)
    s_lo = np.clip(np.ceil(lo2 - WMARGIN), 0, NS - NP2).astype(np.int64)

    Dw = _round_f32r((0.5 * D2[0]).astype(np.float32)).astype(np.float64)  # (e,s)
    wb2 = w2b[0, :]                                       # (e,)
    w2r_np = np.zeros((NP2 + 1, 128), dtype=np.float32)
    q4 = wb2 * c2v[4]
    q3 = np.full(128, 0.0) + wb2 * c2v[3]
    q2 = wb2 * c2v[2] + 0.0
    q1 = wb2 * c2v[1] + 0.0
    q0 = wb2 * c2v[0] + 0.0
    q3 = q3.astype(np.float64); q2 = q2.astype(np.float64)
    q1 = q1.astype(np.float64); q0 = q0.astype(np.float64)
    for e in range(128):
        w_lo = s_lo[e]
        for s in range(NS):
            Ds = Dw[e, s]   # ~ 0.5 * D_full (f32r-rounded)
            c = s - VC2
            if s < w_lo:
                # always below u: relu^3 = (u-s)^3 exactly (weight D_full=2*Ds)
                q3[e] += 2 * Ds
                q2[e] += -3 * c * 2 * Ds
                q1[e] += 3 * c * c * 2 * Ds
                q0[e] += -c ** 3 * 2 * Ds
            elif s < w_lo + NP2:
                # in-window: page gives Ds*(|d|^3 - g2 d^2);
                # fold Ds*(u-s)^3 + Ds*g2*(u-s)^2
                i = s - w_lo
                w2r_np[i, e] = Ds
                q3[e] += Ds
                q2[e] += Ds * (-3 * c + g2)
                q1[e] += Ds * (3 * c * c - 2 * g2 * c)
                q0[e] += Ds * (-c ** 3 + g2 * c * c)
            # s >= w_lo + NP2: always above u: relu = 0, drop
    w2r_np[NP2] = q1
    w2h_np = np.empty((3, 128), dtype=np.float32)
    w2h_np[0] = q2
    w2h_np[1] = q3
    w2h_np[2] = q4

    vecs_np = np.zeros((5, 128), dtype=np.float32)
    vecs_np[0] = s_lo - VC2                    # s02: kan2 page start (v2-basis)
    vecs_np[1] = bv - VC1                      # cp1: v1 bias
    vecs_np[2] = g1
    vecs_np[3] = (c0v - G0) * INV_H - VC2      # cp2: v2 bias (incl c0v fold)
    vecs_np[4] = g2
    q0_sum = float(q0.sum())

    weights = dict(
        wft=wft_np,
        w1h=np.ascontiguousarray(w1h_np.transpose(1, 0, 2)).astype(np.float16),
        w1v=np.ascontiguousarray(w1v_np.transpose(1, 0, 2)).astype(np.float32),
        w2r=np.ascontiguousarray(w2r_np.T).astype(np.float32),
        w2h=np.ascontiguousarray(w2h_np.T).astype(np.float16),
        vecs=np.ascontiguousarray(vecs_np.T).astype(np.float32),
    )
    return stmT, nstmT, weights, q0_sum


def _tile_input(xT_core):
    # (768, BC) -> (128, NBT, KT_FT, NT): [p, bt, k, n] = xT[k*128+p, bt*NT+n]
    return np.ascontiguousarray(
        xT_core.reshape(KT_FT, 128, NBT, NT).transpose(1, 2, 0, 3)
    )


def kernel(**inputs):
    from concourse.bass_utils import run_bass_kernel_spmd

    nc = _build_module()
    stmT, nstmT, weights, q0_sum = _host_prep(inputs)

    in_maps = []
    for c in range(NCORES):
        sl = slice(c * BC, (c + 1) * BC)
        m = {
            "stm_t": _tile_input(stmT[:, sl]),
            "nstm_t": _tile_input(nstmT[:, sl]),
        }
        m.update(weights)
        in_maps.append(m)

    res = run_bass_kernel_spmd(nc, in_maps, core_ids=list(range(NCORES)))
    logits = np.concatenate(
        [r["out"].reshape(-1) for r in res.results]
    ) + q0_sum
    out = 1.0 / (1.0 + np.exp(-logits.astype(np.float64)))
    return out.reshape(B, 1).astype(np.float32)


if __name__ == "__main__":
    rng = np.random.default_rng(0)
    fake = {
        "stm": rng.random((B, IN_FT), dtype=np.float32),
        "nstm": rng.random((B, IN_FT), dtype=np.float32),
        "ft_w": (rng.standard_normal((HID, IN_FT)) * 0.02).astype(np.float32),
        "ft_b": np.zeros(HID, np.float32),
        "kan1_base_w": (rng.standard_normal((HID, 2 * HID)) * 0.05).astype(np.float32),
        "kan1_spline_w": (rng.standard_normal((HID, 2 * HID, 8)) * 0.05).astype(np.float32),
        "kan2_base_w": (rng.standard_normal((1, HID)) * 0.05).astype(np.float32),
        "kan2_spline_w": (rng.standard_normal((1, HID, 8)) * 0.05).astype(np.float32),
    }
    out = kernel(**fake)
    print("kernel out", out.shape, out.dtype, out[:5, 0])
